# revision 14
# baseline (speedup 1.0000x reference)
"""Koopman operator propagation kernel for Trainium2 (Bass/Tile), 8 NeuronCores.

Computes z_{t+1} = z + DT*(z @ A.T + sum_l a_l * U_l (V_l^T z)) for `steps`
steps, data-parallel over the flattened batch dim (262144 rows -> 32768/core).

Layout: z is feature-major (zT: [256, Nc]); batch rows stream through the PE
array as the moving operand in 512-wide column tiles. Per tile, the z master
lives in PSUM (fp32) across all steps: seeded exactly by one fp32r identity
matmul per 128-row half, then each step accumulates DT*(A z + U (a * V^T z))
via fp8e4 DoubleRow matmuls (K=256 per instruction) for the A and V terms and
bf16 matmuls (K=96) for the U term. The moving operand z is re-quantized each
step to e4m3 at scale 1/64 (weights carry the inverse 64x), so fp8 noise only
touches DT-scaled update terms, never the fp32 z master. Tiles are processed
in groups of 3 (6 PSUM banks of master + 2 proj banks) with weight-major PE
ordering so LDWEIGHTS amortizes and the scalar-engine casts of tile t overlap
matmuls of the other tiles.
"""

import numpy as np

P = 128
M = 256            # latent dim
DA = 6             # action dim
R = 16             # low-rank dim
J = DA * R         # 96 concatenated rank columns
B_FULL = 4096
T_FULL = 64
NFULL = B_FULL * T_FULL   # 262144 flattened rows
NCORES = 8
NC_ROWS = NFULL // NCORES  # 32768 rows per core
NT = 512           # column-tile width (one PSUM bank of fp32)
NTILES = NC_ROWS // NT     # 64
DT = 0.1
B_MAX = 0.3
SW = 64.0          # fp8 weight scale; z moving operand carries 1/SW

GROUPS = [3] * 21 + [1]    # 64 column tiles per core

_CACHE = {}
_LAST_RESULT = None


def _build(steps: int, repeat: int = 1):
    from contextlib import ExitStack

    import concourse.mybir as mybir
    import concourse.tile as tile
    from concourse import bacc

    f32 = mybir.dt.float32
    f32r = mybir.dt.float32r
    bf16 = mybir.dt.bfloat16
    f8 = mybir.dt.float8e4
    mult = mybir.AluOpType.mult
    DR = mybir.MatmulPerfMode.DoubleRow
    CopyF = mybir.ActivationFunctionType.Copy

    nc = bacc.Bacc("TRN2", target_bir_lowering=False, num_devices=NCORES)
    zT = nc.declare_dram_parameter("zT", [M, NC_ROWS], f32r, isOutput=False)
    z8T = nc.declare_dram_parameter("z8T", [M, NC_ROWS], f8, isOutput=False)
    aexp = nc.declare_dram_parameter("aexp", [J, NC_ROWS], bf16, isOutput=False)
    wA8 = nc.declare_dram_parameter("wA8", [P, 2, M], f8, isOutput=False)
    wV8 = nc.declare_dram_parameter("wV8", [P, 2, J], f8, isOutput=False)
    wU = nc.declare_dram_parameter("wU", [J, M], bf16, isOutput=False)
    ident = nc.declare_dram_parameter("ident", [P, P], f32r, isOutput=False)
    zO = nc.declare_dram_parameter("zO", [M, NC_ROWS], f32, isOutput=True)

    zr = zT[:].rearrange("(kc p) n -> p kc n", p=P)
    z8r = z8T[:].rearrange("(kc p) n -> p kc n", p=P)
    zOr = zO[:].rearrange("(kc p) n -> p kc n", p=P)

    with tile.TileContext(nc) as tc, ExitStack() as ctx:
        wpool = ctx.enter_context(tc.tile_pool(name="w", bufs=1))
        sdpool = ctx.enter_context(tc.tile_pool(name="zsd", bufs=6))
        z8pool = ctx.enter_context(tc.tile_pool(name="z8", bufs=8))
        apool = ctx.enter_context(tc.tile_pool(name="a", bufs=6))
        ppool = ctx.enter_context(tc.tile_pool(name="proj", bufs=4))
        opool = ctx.enter_context(tc.tile_pool(name="o", bufs=4))
        psz = ctx.enter_context(tc.tile_pool(name="psz", bufs=3, space="PSUM"))
        psp = ctx.enter_context(tc.tile_pool(name="psp", bufs=2, space="PSUM"))

        wa8 = wpool.tile([P, 2, M], f8)
        nc.sync.dma_start(wa8[:], wA8[:])
        wv8 = wpool.tile([P, 2, J], f8)
        nc.sync.dma_start(wv8[:], wV8[:])
        wu = wpool.tile([J, M], bf16)
        nc.sync.dma_start(wu[:], wU[:])
        idt = wpool.tile([P, P], f32r)
        nc.sync.dma_start(idt[:], ident[:])
        idtr = idt[:]

        for _rep in range(repeat):
            _emit_body(nc, mybir, steps, wa8, wv8, wu, idtr,
                       zr, z8r, zOr, aexp, sdpool, z8pool, apool, ppool,
                       opool, psz, psp)
    nc.finalize()
    return nc


def _emit_body(nc, mybir, steps, wa8, wv8, wu, idtr, zr, z8r, zOr, aexp,
               sdpool, z8pool, apool, ppool, opool, psz, psp):
        f32 = mybir.dt.float32
        f32r = mybir.dt.float32r
        bf16 = mybir.dt.bfloat16
        f8 = mybir.dt.float8e4
        mult = mybir.AluOpType.mult
        DR = mybir.MatmulPerfMode.DoubleRow
        CopyF = mybir.ActivationFunctionType.Copy

        tile_idx = 0
        for gsize in GROUPS:
            tiles = []
            for t in range(gsize):
                n0 = (tile_idx + t) * NT
                zsd = sdpool.tile([P, 2, NT], f32r, tag="zsd")
                z8 = z8pool.tile([P, 2, NT], f8, tag="z8")
                for c in (0, 1):
                    nc.sync.dma_start(zsd[:, c, :], zr[:, c, n0:n0 + NT])
                    nc.sync.dma_start(z8[:, c, :], z8r[:, c, n0:n0 + NT])
                at = apool.tile([J, NT], bf16, tag="at")
                nc.sync.dma_start(at[:], aexp[:, n0:n0 + NT])
                pz = [
                    psz.tile([P, NT], f32, tag=f"pz{c}", name=f"pz{c}")
                    for c in (0, 1)
                ]
                tiles.append({"n0": n0, "zsd": zsd, "z8": z8, "a": at, "pz": pz})
            tile_idx += gsize

            # Seed the PSUM master with fp32r identity matmuls (exact to
            # ~fp22): one matmul per 128-row half.
            for tl in tiles:
                for c in (0, 1):
                    nc.tensor.matmul(
                        tl["pz"][c][:], idtr,
                        tl["zsd"][:, c, :],
                        start=True, stop=False, skip_group_check=True,
                    )

            for s in range(steps):
                last = s == steps - 1
                pps = {}
                projs = {}
                znew = {}

                def do_V(t):
                    pp = psp.tile([J, NT], f32, tag="pp")
                    nc.tensor.matmul(
                        pp[:], wv8[:], tiles[t]["z8"][:],
                        start=True, stop=True, perf_mode=DR,
                    )
                    pps[t] = pp

                def do_mult(t):
                    pr = ppool.tile([J, NT], bf16, tag="projs")
                    nc.vector.tensor_tensor(pr[:], pps[t][:], tiles[t]["a"][:], mult)
                    projs[t] = pr

                def do_A(c, t):
                    nc.tensor.matmul(
                        tiles[t]["pz"][c][:],
                        wa8[:, :, c * P:(c + 1) * P],
                        tiles[t]["z8"][:],
                        start=False, stop=False, perf_mode=DR,
                        skip_group_check=True,
                    )

                def do_U(c, t):
                    nc.tensor.matmul(
                        tiles[t]["pz"][c][:],
                        wu[:, c * P:(c + 1) * P],
                        projs[t][:],
                        start=False, stop=last, skip_group_check=True,
                    )

                # PE order (G=3): V0 V1 A00 A01 A02 V2 A10 A11 A12
                #                 U00 U01 U02 U10 U11 U12
                # pp is double-buffered, so V2 sits after the A(c0) block
                # to clear the WAR on pp buffer 0 (read by mult(0)).
                do_V(0)
                if gsize > 1:
                    do_V(1)
                do_mult(0)
                for t in range(gsize):
                    do_A(0, t)
                if gsize > 2:
                    do_V(2)
                if gsize > 1:
                    do_mult(1)
                for t in range(gsize):
                    do_A(1, t)
                if gsize > 2:
                    do_mult(2)

                if not last:
                    for t in range(gsize):
                        znew[t] = z8pool.tile(
                            [P, 2, NT], f8, tag="z8", name=f"znew{t}"
                        )
                    for t in range(gsize):
                        do_U(0, t)
                    for t in range(gsize):
                        nc.scalar.mul(
                            znew[t][:, 0, :], tiles[t]["pz"][0][:], 1.0 / SW
                        )
                    for t in range(gsize):
                        do_U(1, t)
                    for t in range(gsize):
                        nc.scalar.mul(
                            znew[t][:, 1, :], tiles[t]["pz"][1][:], 1.0 / SW
                        )
                    for t in range(gsize):
                        tiles[t]["z8"] = znew[t]
                else:
                    zouts = {
                        t: opool.tile(
                            [P, 2, NT], f32, tag="zout", name=f"zout{t}"
                        )
                        for t in range(gsize)
                    }
                    for t in range(gsize):
                        do_U(0, t)
                    # balance the fp32 evacuation: DVE takes c0 of t0/t1,
                    # ACT takes the rest.
                    for t in range(gsize):
                        if t < 2:
                            nc.vector.tensor_copy(
                                out=zouts[t][:, 0, :], in_=tiles[t]["pz"][0][:]
                            )
                        else:
                            nc.scalar.activation(
                                zouts[t][:, 0, :], tiles[t]["pz"][0][:], CopyF
                            )
                    for t in range(gsize):
                        do_U(1, t)
                    for t in range(gsize):
                        nc.scalar.activation(
                            zouts[t][:, 1, :], tiles[t]["pz"][1][:], CopyF
                        )
                    for t in range(gsize):
                        n0 = tiles[t]["n0"]
                        for c in (0, 1):
                            nc.sync.dma_start(
                                zOr[:, c, n0:n0 + NT], zouts[t][:, c, :]
                            )


def _prep_weights(A, B_U, B_V):
    """Fold DT, tanh clamp, and the fp8 scale into weight tiles (host f64)."""
    import ml_dtypes

    bf = ml_dtypes.bfloat16
    f8 = ml_dtypes.float8_e4m3
    A64 = np.asarray(A, np.float64)
    Uc = np.tanh(np.asarray(B_U, np.float64)) * B_MAX   # (6, 256, 16)
    Vc = np.tanh(np.asarray(B_V, np.float64)) * B_MAX
    # wA8[p, i, mo] = SW * DT * A[mo, i*128+p]
    wA8 = np.ascontiguousarray(
        (SW * DT * A64).T.reshape(2, P, M).transpose(1, 0, 2)
    )
    # wV8[p, i, j] = SW * Vcat[i*128+p, j],  Vcat[k, l*16+r] = Vc[l, k, r]
    Vcat = Vc.transpose(1, 0, 2).reshape(M, J)
    wV8 = np.ascontiguousarray(
        (SW * Vcat).reshape(2, P, J).transpose(1, 0, 2)
    )
    # wU[l*16+r, mo] = DT * Uc[l, mo, r]
    wU = np.ascontiguousarray(
        DT * Uc.transpose(0, 2, 1).reshape(J, M)
    ).astype(bf)
    clip = 240.0
    wA8 = np.clip(wA8, -clip, clip).astype(f8)
    wV8 = np.clip(wV8, -clip, clip).astype(f8)
    return wA8, wV8, wU


def kernel(z, a, A, B_U, B_V, steps):
    from concourse.bass_utils import run_bass_kernel_spmd

    steps = int(steps)
    z = np.asarray(z, np.float32)
    out_shape = z.shape
    if steps == 0:
        return z.copy()

    if (steps, 1) not in _CACHE:
        _CACHE[(steps, 1)] = _build(steps)
    nc = _CACHE[(steps, 1)]

    in_maps = make_in_maps(z, a, A, B_U, B_V)
    res = run_bass_kernel_spmd(nc, in_maps, core_ids=list(range(NCORES)))
    global _LAST_RESULT
    _LAST_RESULT = res
    zo = np.concatenate([res.results[c]["zO"] for c in range(NCORES)], axis=1)
    return np.ascontiguousarray(zo.T).reshape(out_shape)


def make_in_maps(z, a, A, B_U, B_V):
    """Host-side input prep, shared by kernel() and the timing harness."""
    import ml_dtypes

    bf = ml_dtypes.bfloat16
    f8 = ml_dtypes.float8_e4m3
    z_f = np.asarray(z, np.float32).reshape(-1, M)
    a_f = np.asarray(a, np.float32).reshape(-1, DA)
    wA8, wV8, wU = _prep_weights(A, B_U, B_V)
    ident = np.eye(P, dtype=np.float32)
    zT = np.ascontiguousarray(z_f.T)
    z8 = (zT * np.float32(1.0 / SW)).astype(f8)
    aex = np.ascontiguousarray(np.repeat(a_f.T, R, axis=0).astype(bf))
    in_maps = []
    for c in range(NCORES):
        sl = slice(c * NC_ROWS, (c + 1) * NC_ROWS)
        in_maps.append(
            {
                "zT": np.ascontiguousarray(zT[:, sl]),
                "z8T": np.ascontiguousarray(z8[:, sl]),
                "aexp": np.ascontiguousarray(aex[:, sl]),
                "wA8": wA8,
                "wV8": wV8,
                "wU": wU,
                "ident": ident,
            }
        )
    return in_maps
